# revision 6
# baseline (speedup 1.0000x reference)
"""Mixture-of-Experts (E=8, top-2) — F-sliced Trainium2 Bass kernel.

Strategy (intermediate-dim sharding; perfectly load-balanced):
  * Host computes the router (logits -> top-2 -> softmax) in numpy and sorts
    the 2*T (token, slot) pairs by expert.
  * Core c keeps ALL 8 experts' weights resident, but only the F-column slice
    [512c, 512(c+1)) of each — 16.8 MB of bf16, fits SBUF.  Every core streams
    ALL pairs through its slice:  y_part = w2[e][:, fs].T' @ gelu(w1[e][fs] @ x
    + b1[fs]).  Partials are evicted in bf16 and summed on the host (+ b2 and
    the top-2 prob combine).
  * Because every core runs every pair, the work is identical on all cores no
    matter how tokens route: 16384 matmul columns each, zero capacity padding.
    The chunk schedule (chunks never straddle an expert boundary) is baked
    into the program from the exact per-expert counts.

Device layout (per core, SPMD — same program, per-core weight slices):
  xt   [D, TP]      bf16  all pairs, expert-sorted, token dim = free dim
  w1s  [E, D, FS]   bf16  w1[e].T column-slice   (contract D on partitions)
  w2s  [E, FS, D]   bf16  w2[e].T row-slice      (contract FS on partitions)
  b1s  [P, E, MF]   f32   b1 slice as per-partition bias table
  yt   [D, TP]      bf16  partial expert outputs (summed across cores on host)
"""

import numpy as np
from contextlib import ExitStack

from ml_dtypes import bfloat16

import concourse.bacc as bacc
import concourse.tile as tile
import concourse.mybir as mybir
from concourse.bass_utils import run_bass_kernel_spmd

P = 128
D = 1024
F = 4096
E = 8
TOPK = 2
B, S = 4, 2048
T = B * S
TP = TOPK * T      # 16384 (token, slot) pairs, each a matmul column

FS = F // E        # 512  F-slice width per core
NT = 512           # max tokens per chunk (matmul moving free dim limit)

KD = D // P        # 8  k-tiles for MLP1 (contract D)
MF = FS // P       # 4  m-tiles for MLP1 output (F slice)
KS = FS // P       # 4  k-tiles for MLP2 (contract F slice)
MD = D // P        # 8  m-tiles for MLP2 output (D)

_prog_cache: dict = {}
ACT_FUNC = None  # default: Gelu; sim_check overrides (CoreSim lacks Gelu)
last_results = None  # BassKernelResults of the most recent run (for test harness)
trace_kwargs: dict = {}  # test harness can set e.g. {"trace": True}


def _split_even(total, lead=0):
    """Split `total` cols into near-equal chunks <= NT (plus an optional small
    leading chunk).  Equal widths keep every matmul >= ~128 cols so the PE's
    weight preload stays hidden; a 512*k + tiny-remainder split does not."""
    sizes = []
    if lead and total > lead:
        sizes.append(lead)
        total -= lead
    k = -(-total // NT)
    base, extra = divmod(total, k)
    sizes.extend([base + 1] * extra + [base] * (k - extra))
    return sizes


def _schedule(counts):
    """Chunk schedule [(expert, col_offset, width), ...] — no chunk straddles
    an expert boundary; widths <= NT; total width == sum(counts).

    The first chunk is small so the PE can start as soon as ~0.5 MB of DMA
    lands; the last chunk is small so the MLP2+evict+writeback drain after the
    final MLP1 is short."""
    sched = []
    off = 0
    last_e = max((e for e in range(E) if counts[e] > 0), default=0)
    for e in range(E):
        if counts[e] == 0:
            continue
        sizes = _split_even(int(counts[e]), lead=128 if not sched else 0)
        if e == last_e and sizes[-1] > 256:
            sizes = sizes[:-1] + [sizes[-1] - 128, 128]
        for n in sizes:
            sched.append((e, off, n))
            off += n
    return tuple(sched)


def _build_program(sched):
    """Build + compile the SPMD F-sliced all-experts program."""
    bf16 = mybir.dt.bfloat16
    f32 = mybir.dt.float32

    nc = bacc.Bacc(
        "TRN2",
        target_bir_lowering=False,
        debug=False,
        enable_asserts=False,
        num_devices=E,
    )

    xt = nc.dram_tensor("xt", [D, TP], bf16, kind="ExternalInput").ap()
    w1s = nc.dram_tensor("w1s", [E, D, FS], bf16, kind="ExternalInput").ap()
    w2s = nc.dram_tensor("w2s", [E, FS, D], bf16, kind="ExternalInput").ap()
    b1s = nc.dram_tensor("b1s", [P, E, MF], f32, kind="ExternalInput").ap()
    yt = nc.dram_tensor("yt", [D, TP], bf16, kind="ExternalOutput").ap()

    # Partition-tiled DRAM views (one multi-dim AP DMA instead of many
    # row-block DMAs; DMA issue costs ~650ns of engine time each).
    xt_r = xt.rearrange("(k p) t -> p k t", p=P)      # [128, KD, TP]
    w1s_r = w1s.rearrange("e (k p) f -> p e k f", p=P)  # [128, E, KD, FS]
    w2s_r = w2s.rearrange("e (k p) d -> p e k d", p=P)  # [128, E, KS, D]
    yt_r = yt.rearrange("(m p) t -> p m t", p=P)      # [128, MD, TP]

    with tile.TileContext(nc) as tc, ExitStack() as ctx:
        wpool = ctx.enter_context(tc.tile_pool(name="wpool", bufs=1))
        xpool = ctx.enter_context(tc.tile_pool(name="xpool", bufs=3))
        hpool = ctx.enter_context(tc.tile_pool(name="hpool", bufs=2))
        ypool = ctx.enter_context(tc.tile_pool(name="ypool", bufs=3))
        ps1 = ctx.enter_context(tc.tile_pool(name="ps1", bufs=3, space="PSUM"))
        ps2 = ctx.enter_context(tc.tile_pool(name="ps2", bufs=3, space="PSUM"))

        # Expert 0's weights are latency-critical (compute reaches expert e's
        # segment ~55us*e in, but expert 0 is needed ~1us in).  The gpsimd
        # SWDGE queue takes ~10us to start moving data, so expert 0 goes in
        # small pieces on the scalar HWDGE queue (idle until the first y
        # writeback ~10us in), in consumption order: w1[0] by m-tile, then
        # w2[0] in halves.  sync carries b1 then the x chunks; gpsimd carries
        # experts 1..7 (needed ~55us+ in — plenty of time).
        b1_sb = wpool.tile([P, E, MF], f32, name="b1sb")
        nc.sync.dma_start(out=b1_sb[:, :, :], in_=b1s[:, :, :])
        w1_sb = [wpool.tile([P, KD, FS], bf16, name=f"w1_{e}") for e in range(E)]
        w2_sb = [wpool.tile([P, KS, D], bf16, name=f"w2_{e}") for e in range(E)]
        for m in range(MF):
            nc.scalar.dma_start(
                out=w1_sb[0][:, :, m * P : (m + 1) * P],
                in_=w1s_r[:, 0, :, m * P : (m + 1) * P],
            )
        for h in range(2):
            nc.scalar.dma_start(
                out=w2_sb[0][:, :, h * (D // 2) : (h + 1) * (D // 2)],
                in_=w2s_r[:, 0, :, h * (D // 2) : (h + 1) * (D // 2)],
            )
        for e in range(1, E):
            nc.gpsimd.dma_start(out=w1_sb[e][:, :, :], in_=w1s_r[:, e])
            nc.gpsimd.dma_start(out=w2_sb[e][:, :, :], in_=w2s_r[:, e])

        for e, off, n in sched:
            x_sb = xpool.tile([P, KD, NT], bf16, name="xtile")
            nc.sync.dma_start(out=x_sb[:, :, :n], in_=xt_r[:, :, off : off + n])

            # MLP1: h[FS, n] = gelu(w1s[e].T @ x + b1s[e]), bf16 out
            h_sb = hpool.tile([P, KS, NT], bf16, name="htile")
            for m in range(MF):
                pt = ps1.tile([P, NT], f32, name="p1")
                for k in range(KD):
                    nc.tensor.matmul(
                        pt[:, :n],
                        lhsT=w1_sb[e][:, k, m * P : (m + 1) * P],
                        rhs=x_sb[:, k, :n],
                        start=(k == 0),
                        stop=(k == KD - 1),
                    )
                nc.scalar.activation(
                    h_sb[:, m, :n],
                    pt[:, :n],
                    ACT_FUNC or mybir.ActivationFunctionType.Gelu,
                    bias=b1_sb[:, e, m : m + 1],
                )

            # MLP2 partial: y[D, n] = w2s[e].T @ h, bf16 out (b2 on host)
            y_sb = ypool.tile([P, MD, NT], bf16, name="ytile")
            for m in range(MD):
                pt = ps2.tile([P, NT], f32, name="p2")
                for k in range(KS):
                    nc.tensor.matmul(
                        pt[:, :n],
                        lhsT=w2_sb[e][:, k, m * P : (m + 1) * P],
                        rhs=h_sb[:, k, :n],
                        start=(k == 0),
                        stop=(k == KS - 1),
                    )
                nc.vector.tensor_copy(out=y_sb[:, m, :n], in_=pt[:, :n])
                if m == MD // 2 - 1 or m == MD - 1:
                    h0 = m + 1 - MD // 2
                    nc.scalar.dma_start(
                        out=yt_r[:, h0 : m + 1, off : off + n],
                        in_=y_sb[:, h0 : m + 1, :n],
                    )

    nc.compile()
    return nc


def _get_program(sched):
    if sched not in _prog_cache:
        _prog_cache[sched] = _build_program(sched)
    return _prog_cache[sched]


def _route(xf: np.ndarray, router_w: np.ndarray):
    """Top-2 routing identical to the reference (ties -> lower expert idx).

    Logits in fp64 so the selection is independent of BLAS blocking/threads
    (top-2 gaps in this regime are >= ~3e-6; fp64 noise is ~1e-15).
    """
    logits = xf.astype(np.float64) @ router_w.T.astype(np.float64)  # [T, E]
    idx = np.argsort(-logits, axis=1, kind="stable")[:, :TOPK]
    vals = np.take_along_axis(logits, idx, axis=1)
    vals = vals - vals.max(axis=1, keepdims=True)
    ev = np.exp(vals)
    probs = (ev / ev.sum(axis=1, keepdims=True)).astype(np.float32)
    return idx.astype(np.int64), probs


def kernel(x, router_w, w1, b1, w2, b2):
    global last_results

    x = np.asarray(x, dtype=np.float32)
    router_w = np.asarray(router_w, dtype=np.float32)
    w1 = np.asarray(w1, dtype=np.float32)
    b1 = np.asarray(b1, dtype=np.float32)
    w2 = np.asarray(w2, dtype=np.float32)
    b2 = np.asarray(b2, dtype=np.float32)

    orig_shape = x.shape
    xf = x.reshape(-1, D)

    idx, probs = _route(xf, router_w)

    # Group the (token, k) pairs by expert; gpos = column in the sorted order.
    flat_e = idx.ravel()  # entry j corresponds to token j//2, slot j%2
    order = np.argsort(flat_e, kind="stable")
    counts = np.bincount(flat_e, minlength=E)
    starts = np.zeros(E + 1, dtype=np.int64)
    np.cumsum(counts, out=starts[1:])
    rank = np.empty(TP, dtype=np.int64)
    rank[order] = np.arange(TP, dtype=np.int64) - starts[flat_e[order]]
    gpos = (rank + starts[flat_e]).reshape(T, TOPK)

    nc = _get_program(_schedule(counts))

    xt = np.ascontiguousarray(xf.astype(bfloat16)[order // 2].T)  # [D, TP]
    in_maps = []
    for c in range(E):
        fs = slice(c * FS, (c + 1) * FS)
        w1c = np.ascontiguousarray(
            w1[:, fs, :].transpose(0, 2, 1)
        ).astype(bfloat16)                                        # [E, D, FS]
        w2c = np.ascontiguousarray(
            w2[:, :, fs].transpose(0, 2, 1)
        ).astype(bfloat16)                                        # [E, FS, D]
        b1c = np.ascontiguousarray(
            b1[:, fs].reshape(E, MF, P).transpose(2, 0, 1)
        )                                                         # [P, E, MF]
        in_maps.append({"xt": xt, "w1s": w1c, "w2s": w2c, "b1s": b1c})

    res = run_bass_kernel_spmd(nc, in_maps, core_ids=list(range(E)), **trace_kwargs)
    last_results = res

    acc = np.zeros((D, TP), dtype=np.float32)
    for r in res.results:
        acc += np.asarray(r["yt"]).astype(np.float32)
    accT = acc.T                                                  # [TP, D]
    out = probs[:, 0:1] * (accT[gpos[:, 0]] + b2[idx[:, 0]])
    out += probs[:, 1:2] * (accT[gpos[:, 1]] + b2[idx[:, 1]])
    return out.astype(np.float32).reshape(orig_shape)


# revision 7
# speedup vs baseline: 1.0641x; 1.0641x over previous
"""Mixture-of-Experts (E=8, top-2) — F-sliced Trainium2 Bass kernel.

Strategy (intermediate-dim sharding; perfectly load-balanced):
  * Host computes the router (logits -> top-2 -> softmax) in numpy and sorts
    the 2*T (token, slot) pairs by expert.
  * Core c keeps ALL 8 experts' weights resident, but only the F-column slice
    [512c, 512(c+1)) of each — 16.8 MB of bf16, fits SBUF.  Every core streams
    ALL pairs through its slice:  y_part = w2[e][:, fs].T' @ gelu(w1[e][fs] @ x
    + b1[fs]).  Partials are evicted in bf16 and summed on the host (+ b2 and
    the top-2 prob combine).
  * Because every core runs every pair, the work is identical on all cores no
    matter how tokens route: 16384 matmul columns each, zero capacity padding.
    The chunk schedule (chunks never straddle an expert boundary) is baked
    into the program from the exact per-expert counts.

DMA layout note: the HW DGE queues are packet-rate limited (~110 packets/us),
so every DRAM-side access pattern here is packed host-side to be ONE
contiguous run per partition (128 packets per transfer, ~8 KB each) — flat
chunk-major x/y streams, partition-major weights.

Device layout (per core, SPMD — same program, per-core weight slices):
  xtf  [P, KD*TP]    bf16  chunk-major packed tokens (see _pack_x)
  w1s  [E, P, KD*FS] bf16  partition-major w1[e].T column-slice
  w2s  [E, P, KS*D]  bf16  partition-major w2[e].T row-slice
  b1s  [P, E*MF]     f32   b1 slice as per-partition bias table
  ytf  [P, MD*TP]    bf16  chunk-major packed partial outputs
"""

import numpy as np
from contextlib import ExitStack

from ml_dtypes import bfloat16

import concourse.bacc as bacc
import concourse.tile as tile
import concourse.mybir as mybir
from concourse.bass_utils import run_bass_kernel_spmd

P = 128
D = 1024
F = 4096
E = 8
TOPK = 2
B, S = 4, 2048
T = B * S
TP = TOPK * T      # 16384 (token, slot) pairs, each a matmul column

FS = F // E        # 512  F-slice width per core
NT = 512           # max tokens per chunk (matmul moving free dim limit)

KD = D // P        # 8  k-tiles for MLP1 (contract D)
MF = FS // P       # 4  m-tiles for MLP1 output (F slice)
KS = FS // P       # 4  k-tiles for MLP2 (contract F slice)
MD = D // P        # 8  m-tiles for MLP2 output (D)

_prog_cache: dict = {}
ACT_FUNC = None  # default: Gelu; sim_check overrides (CoreSim lacks Gelu)
last_results = None  # BassKernelResults of the most recent run (for test harness)
trace_kwargs: dict = {}  # test harness can set e.g. {"trace": True}


def _split_even(total, lead=0):
    """Split `total` cols into near-equal chunks <= NT (plus an optional small
    leading chunk).  Equal widths keep every matmul >= ~128 cols so the PE's
    weight preload stays hidden; a 512*k + tiny-remainder split does not."""
    sizes = []
    if lead and total > lead:
        sizes.append(lead)
        total -= lead
    k = -(-total // NT)
    base, extra = divmod(total, k)
    sizes.extend([base + 1] * extra + [base] * (k - extra))
    return sizes


def _schedule(counts):
    """Chunk schedule [(expert, col_offset, width), ...] — no chunk straddles
    an expert boundary; widths <= NT; total width == sum(counts).

    The first chunk is small so the PE can start as soon as ~0.5 MB of DMA
    lands; the last chunk is small so the MLP2+evict+writeback drain after the
    final MLP1 is short."""
    sched = []
    off = 0
    last_e = max((e for e in range(E) if counts[e] > 0), default=0)
    for e in range(E):
        if counts[e] == 0:
            continue
        sizes = _split_even(int(counts[e]), lead=128 if not sched else 0)
        if e == last_e and sizes[-1] > 256:
            sizes = sizes[:-1] + [sizes[-1] - 128, 128]
        for n in sizes:
            sched.append((e, off, n))
            off += n
    return tuple(sched)


def _build_program(sched):
    """Build + compile the SPMD F-sliced all-experts program."""
    bf16 = mybir.dt.bfloat16
    f32 = mybir.dt.float32

    nc = bacc.Bacc(
        "TRN2",
        target_bir_lowering=False,
        debug=False,
        enable_asserts=False,
        num_devices=E,
    )

    xtf = nc.dram_tensor("xtf", [P, KD * TP], bf16, kind="ExternalInput").ap()
    w1s = nc.dram_tensor("w1s", [E, P, KD * FS], bf16, kind="ExternalInput").ap()
    w2s = nc.dram_tensor("w2s", [E, P, KS * D], bf16, kind="ExternalInput").ap()
    b1s = nc.dram_tensor("b1s", [P, E * MF], f32, kind="ExternalInput").ap()
    ytf = nc.dram_tensor("ytf", [P, MD * TP], bf16, kind="ExternalOutput").ap()

    with tile.TileContext(nc) as tc, ExitStack() as ctx:
        wpool = ctx.enter_context(tc.tile_pool(name="wpool", bufs=1))
        xpool = ctx.enter_context(tc.tile_pool(name="xpool", bufs=3))
        hpool = ctx.enter_context(tc.tile_pool(name="hpool", bufs=2))
        ypool = ctx.enter_context(tc.tile_pool(name="ypool", bufs=3))
        ps1 = ctx.enter_context(tc.tile_pool(name="ps1", bufs=3, space="PSUM"))
        ps2 = ctx.enter_context(tc.tile_pool(name="ps2", bufs=3, space="PSUM"))

        # Expert 0's weights are latency-critical (compute reaches expert e's
        # segment ~55us*e in, but expert 0 is needed right away).  The gpsimd
        # SWDGE queue takes ~11us before data moves, so expert 0 rides the
        # scalar HWDGE queue (idle until the first y writeback ~15us in).
        # sync carries b1 then the x chunks; gpsimd carries experts 1..7.
        b1_sb = wpool.tile([P, E, MF], f32, name="b1sb")
        nc.sync.dma_start(out=b1_sb[:, :, :], in_=b1s.rearrange("p (e m) -> p e m", e=E))
        w1_sb = [wpool.tile([P, KD, FS], bf16, name=f"w1_{e}") for e in range(E)]
        w2_sb = [wpool.tile([P, KS, D], bf16, name=f"w2_{e}") for e in range(E)]

        def load_w(e, eng):
            eng.dma_start(
                out=w1_sb[e][:, :, :],
                in_=w1s[e].rearrange("p (k f) -> p k f", k=KD),
            )
            eng.dma_start(
                out=w2_sb[e][:, :, :],
                in_=w2s[e].rearrange("p (k d) -> p k d", k=KS),
            )

        load_w(0, nc.scalar)
        for e in range(1, E):
            load_w(e, nc.gpsimd)

        for e, off, n in sched:
            x_sb = xpool.tile([P, KD * NT], bf16, name="xtile")
            nc.sync.dma_start(
                out=x_sb[:, : KD * n], in_=xtf[:, KD * off : KD * (off + n)]
            )

            # MLP1: h[FS, n] = gelu(w1s[e].T @ x + b1s[e]), bf16 out
            h_sb = hpool.tile([P, KS, NT], bf16, name="htile")
            for m in range(MF):
                pt = ps1.tile([P, NT], f32, name="p1")
                for k in range(KD):
                    nc.tensor.matmul(
                        pt[:, :n],
                        lhsT=w1_sb[e][:, k, m * P : (m + 1) * P],
                        rhs=x_sb[:, k * n : (k + 1) * n],
                        start=(k == 0),
                        stop=(k == KD - 1),
                    )
                nc.scalar.activation(
                    h_sb[:, m, :n],
                    pt[:, :n],
                    ACT_FUNC or mybir.ActivationFunctionType.Gelu,
                    bias=b1_sb[:, e, m : m + 1],
                )

            # MLP2 partial: y[D, n] = w2s[e].T @ h, bf16 out (b2 on host),
            # written back in halves so the drain after the last MLP1 is short
            y_sb = ypool.tile([P, MD * NT], bf16, name="ytile")
            for m in range(MD):
                pt = ps2.tile([P, NT], f32, name="p2")
                for k in range(KS):
                    nc.tensor.matmul(
                        pt[:, :n],
                        lhsT=w2_sb[e][:, k, m * P : (m + 1) * P],
                        rhs=h_sb[:, k, :n],
                        start=(k == 0),
                        stop=(k == KS - 1),
                    )
                nc.vector.tensor_copy(
                    out=y_sb[:, m * n : (m + 1) * n], in_=pt[:, :n]
                )
                if m == MD // 2 - 1 or m == MD - 1:
                    h0 = (m + 1 - MD // 2) * n
                    nc.scalar.dma_start(
                        out=ytf[:, MD * off + h0 : MD * off + (m + 1) * n],
                        in_=y_sb[:, h0 : (m + 1) * n],
                    )

    nc.compile()
    return nc


def _get_program(sched):
    if sched not in _prog_cache:
        _prog_cache[sched] = _build_program(sched)
    return _prog_cache[sched]


def _route(xf: np.ndarray, router_w: np.ndarray):
    """Top-2 routing identical to the reference (ties -> lower expert idx).

    Logits in fp64 so the selection is independent of BLAS blocking/threads
    (top-2 gaps in this regime are >= ~3e-6; fp64 noise is ~1e-15).
    """
    logits = xf.astype(np.float64) @ router_w.T.astype(np.float64)  # [T, E]
    idx = np.argsort(-logits, axis=1, kind="stable")[:, :TOPK]
    vals = np.take_along_axis(logits, idx, axis=1)
    vals = vals - vals.max(axis=1, keepdims=True)
    ev = np.exp(vals)
    probs = (ev / ev.sum(axis=1, keepdims=True)).astype(np.float32)
    return idx.astype(np.int64), probs


def _pack_x(xf_bf, order, sched):
    """Pack gathered tokens chunk-major: chunk (off, n) occupies xtf columns
    [KD*off, KD*(off+n)), laid out [KD, n] so each partition's slice is one
    contiguous 2*KD*n-byte run."""
    xtf = np.empty((P, KD * TP), dtype=bfloat16)
    gathered = xf_bf[order // 2]                      # [TP, D]
    for _, off, n in sched:
        blk = gathered[off : off + n].T               # [D, n]
        blk = blk.reshape(KD, P, n).transpose(1, 0, 2).reshape(P, KD * n)
        xtf[:, KD * off : KD * (off + n)] = blk
    return xtf


def _unpack_y(acc, sched):
    """Inverse of the ytf packing: returns accT [TP, D] (pair-major)."""
    accT = np.empty((TP, D), dtype=np.float32)
    for _, off, n in sched:
        blk = acc[:, MD * off : MD * (off + n)].reshape(P, MD, n)
        accT[off : off + n] = blk.transpose(1, 0, 2).reshape(D, n).T
    return accT


def kernel(x, router_w, w1, b1, w2, b2):
    global last_results

    x = np.asarray(x, dtype=np.float32)
    router_w = np.asarray(router_w, dtype=np.float32)
    w1 = np.asarray(w1, dtype=np.float32)
    b1 = np.asarray(b1, dtype=np.float32)
    w2 = np.asarray(w2, dtype=np.float32)
    b2 = np.asarray(b2, dtype=np.float32)

    orig_shape = x.shape
    xf = x.reshape(-1, D)

    idx, probs = _route(xf, router_w)

    # Group the (token, k) pairs by expert; gpos = column in the sorted order.
    flat_e = idx.ravel()  # entry j corresponds to token j//2, slot j%2
    order = np.argsort(flat_e, kind="stable")
    counts = np.bincount(flat_e, minlength=E)
    starts = np.zeros(E + 1, dtype=np.int64)
    np.cumsum(counts, out=starts[1:])
    rank = np.empty(TP, dtype=np.int64)
    rank[order] = np.arange(TP, dtype=np.int64) - starts[flat_e[order]]
    gpos = (rank + starts[flat_e]).reshape(T, TOPK)

    sched = _schedule(counts)
    nc = _get_program(sched)

    xtf = _pack_x(xf.astype(bfloat16), order, sched)
    in_maps = []
    for c in range(E):
        fs = slice(c * FS, (c + 1) * FS)
        # partition-major: [E, P, KD*FS] with row p = w1[e].T[k*128+p, :]
        w1c = np.ascontiguousarray(
            w1[:, fs, :].transpose(0, 2, 1)        # [E, D, FS]
            .reshape(E, KD, P, FS).transpose(0, 2, 1, 3)
            .reshape(E, P, KD * FS)
        ).astype(bfloat16)
        w2c = np.ascontiguousarray(
            w2[:, :, fs].transpose(0, 2, 1)        # [E, FS, D]
            .reshape(E, KS, P, D).transpose(0, 2, 1, 3)
            .reshape(E, P, KS * D)
        ).astype(bfloat16)
        b1c = np.ascontiguousarray(
            b1[:, fs].reshape(E, MF, P).transpose(2, 0, 1).reshape(P, E * MF)
        )
        in_maps.append({"xtf": xtf, "w1s": w1c, "w2s": w2c, "b1s": b1c})

    res = run_bass_kernel_spmd(nc, in_maps, core_ids=list(range(E)), **trace_kwargs)
    last_results = res

    acc = np.zeros((P, MD * TP), dtype=np.float32)
    for r in res.results:
        acc += np.asarray(r["ytf"]).astype(np.float32)
    accT = _unpack_y(acc, sched)                                  # [TP, D]
    out = probs[:, 0:1] * (accT[gpos[:, 0]] + b2[idx[:, 0]])
    out += probs[:, 1:2] * (accT[gpos[:, 1]] + b2[idx[:, 1]])
    return out.astype(np.float32).reshape(orig_shape)
